# revision 1
# baseline (speedup 1.0000x reference)
"""Trainium2 Bass kernel for nn_DiffusionGraphConv_78374563217429.

Math reformulation (exact algebra, verified to 4e-7 vs reference):
  reference out = concat_m(x_m) @ W  with  xs = [x0, A0 x0, 2 A0^2 x0 - x0,
                                                 A1 x0, 2 A1^2 x0 - x0]
  Since everything is linear, push W through the recurrence:
      out = x0 @ Wd + sum_s A_s @ (x0 @ W1s + A_s @ (x0 @ 2 W2s))
  with Wd = W0 - W20 - W21.  This shrinks each SpMM application from 128
  features to 64 and removes the big final [B*N,640]@[640,64] matmul.

Implementation: the COO supports are densified host-side (static graph
preprocessing) into fp16 [4096,4096] matrices laid out in matmul-ready
panels; on each core the recurrence runs as dense TensorE matmuls with
fp32 PSUM accumulation (global rel err ~3.6e-4 in fp16).

Sharding: data-parallel over batch, 4 batch items per core x 8 cores;
supports/weights replicated.
"""

import os
import sys

import numpy as np

# ---------------------------------------------------------------- constants
P = 128          # partitions
N = 4096         # nodes
NM = 32          # output-node chunks (N / P)
NK = 32          # contraction-node chunks (N / P)
BC = 4           # batch items per core
FREE = BC * 64   # matmul moving free dim for SpMM passes (4 batches x 64 feat)
PW = 320         # P-phase Wcat columns: [u0 | wt0 | init | wt1 | u1] x 64
NCORES = 8

_COMPILED = None     # cached (nc, ) across kernel() calls
LAST_RESULTS = None  # BassKernelResults of the most recent run (for test.py)


def _import_concourse():
    try:
        import concourse.bass  # noqa: F401
    except ImportError:
        for p in ("/opt/trn_rl_repo", "/root/.axon_site/_ro/trn_rl_repo"):
            if os.path.isdir(p) and p not in sys.path:
                sys.path.insert(0, p)
        import concourse.bass  # noqa: F401
    # bass_utils imports antenv.axon_hooks when tracing is requested; some
    # images lack that module — stub it so BASS_TRACE never crashes the run.
    try:
        import antenv.axon_hooks  # noqa: F401
    except ImportError:
        import types
        mod = types.ModuleType("antenv.axon_hooks")
        mod.get_axon_ntff_profile_hook = lambda: None
        mod.set_axon_ntff_profile_hook = lambda h: None
        sys.modules["antenv.axon_hooks"] = mod


def _build_module():
    """Trace the Bass/Tile module (identical SPMD program for all 8 cores)."""
    import concourse.mybir as mybir
    from concourse import bacc
    from concourse.tile import TileContext

    f16 = mybir.dt.float16
    f32 = mybir.dt.float32

    nc = bacc.Bacc("TRN2", target_bir_lowering=False, debug=False,
                   num_devices=NCORES)

    at0 = nc.dram_tensor("at0", [NM, P, NK, P], f16, kind="ExternalInput").ap()
    at1 = nc.dram_tensor("at1", [NM, P, NK, P], f16, kind="ExternalInput").ap()
    x0t = nc.dram_tensor("x0t", [BC, P, N], f16, kind="ExternalInput").ap()
    wcat = nc.dram_tensor("wcat", [P, PW], f16, kind="ExternalInput").ap()
    outd = nc.dram_tensor("out", [P, NM * FREE], f32, kind="ExternalOutput").ap()

    ats = (at0, at1)

    with TileContext(nc) as tc:
        with (
            tc.tile_pool(name="singles", bufs=1) as singles,
            tc.tile_pool(name="uw", bufs=1) as uwpool,
            tc.tile_pool(name="trans", bufs=5) as trans,
            tc.tile_pool(name="xp", bufs=2) as xp,
        ):
            wcat_sb = singles.tile([P, PW], f16, name="wcat_sb")
            nc.sync.dma_start(out=wcat_sb, in_=wcat)

            # ---- PE warmup: HAM clock-gate starts at 1.2 GHz and only
            # releases after ~3.4us of sustained PE activity.  Fill the
            # DMA-load window with dummy matmuls so the real work runs
            # at 2.4 GHz from the first instruction.
            wlhs = singles.tile([P, P], f16, name="wlhs")
            wrhs = singles.tile([P, 512], f16, name="wrhs")
            nc.vector.memset(wlhs, 0.0)
            nc.vector.memset(wrhs, 0.0)

            # persistent SBUF buffers
            # bigp sections: 0=u0, 1=wt0, 2=init, 3=wt1, 4=u1;
            # layout [p, s, m, b*64+f]
            bigp = singles.tile([P, 5, NM, FREE], f16, name="bigp")
            w0_sb = singles.tile([P, NK, FREE], f16, name="w0_sb")
            out_sb = singles.tile([P, NM, FREE], f32, name="out_sb")

            # ---------------- P phase: P_b = x0_b @ Wcat ----------------
            # Wcat col sections: [0:64]=2*W20 (u0), [64:128]=W10 (wt0),
            # [128:192]=Wd (init), [192:256]=W11 (wt1), [256:320]=2*W21 (u1)
            sp_cm = tc.tile_pool(name="sp", bufs=2, space="PSUM")
            sp = sp_cm.__enter__()
            pp_cm = tc.tile_pool(name="pp", bufs=3, space="PSUM")
            pp = pp_cm.__enter__()
            wps = sp.tile([P, FREE], f32, tag="sp_ps", name="warm_ps")
            for _ in range(30):
                nc.tensor.matmul(wps, wlhs, wrhs[:, :FREE], start=True, stop=True)
            for b in range(BC):
                xt = xp.tile([P, N], f16, tag="xt", name="xt")
                # two half-tile DMAs: the first m-chunks' matmuls only
                # depend on the first half, starting the P phase earlier
                nc.sync.dma_start(out=xt[:, :N // 2], in_=x0t[b, :, :N // 2])
                nc.sync.dma_start(out=xt[:, N // 2:], in_=x0t[b, :, N // 2:])
                for mg in range(NM // 2):
                    ps = pp.tile([P, 2, 512], f32, tag="pp_ps", name="pp_ps")
                    for mi in range(2):
                        m = mg * 2 + mi
                        nc.tensor.matmul(
                            ps[:, mi, :PW],
                            xt[:, m * P:(m + 1) * P],
                            wcat_sb,
                            start=True, stop=True,
                        )
                    # single strided evacuation of all 5 sections, groups
                    # alternating DVE/ACT: halves the per-op fixed costs
                    sec_eng = nc.vector.tensor_copy if mg % 2 == 0 else (
                        lambda out, in_: nc.scalar.copy(out=out, in_=in_))
                    sec_eng(
                        out=bigp[:, :, mg * 2:(mg + 1) * 2, b * 64:(b + 1) * 64],
                        in_=ps[:, :, 0:PW].rearrange(
                            "p m (s f) -> p s m f", f=64),
                    )

            # ---------------- SpMM passes ----------------
            def spmm_pass(at_ap, rhs_fn, post_fn, panel_ring=None):
                for m in range(NM):
                    panel = trans.tile([P, NK, P], f16, tag="big8k", name="panel")
                    # alternate HWDGE rings (SP / ACT) so panel loads stream
                    # on both queues instead of one FIFO
                    if panel_ring is None:
                        dma_eng = nc.sync if m % 2 == 0 else nc.scalar
                    else:
                        dma_eng = panel_ring
                    dma_eng.dma_start(out=panel, in_=at_ap[m])
                    ps = sp.tile([P, FREE], f32, name="sp_ps")
                    for kc in range(NK):
                        nc.tensor.matmul(
                            ps,
                            panel[:, kc, :],
                            rhs_fn(kc),
                            start=(kc == 0), stop=(kc == NK - 1),
                        )
                    post_fn(m, ps)

            # v0 = A0 @ u0 ;  w0 = wt0 + v0
            spmm_pass(
                ats[0],
                lambda kc: bigp[:, 0, kc, :],
                lambda m, ps: nc.vector.tensor_add(
                    out=w0_sb[:, m, :], in0=ps, in1=bigp[:, 1, m, :]),
            )
            # v1 = A1 @ u1 ;  w1 = wt1 + v1
            w1_sb = uwpool.tile([P, NK, FREE], f16, tag="uw", name="w1_sb")
            spmm_pass(
                ats[1],
                lambda kc: bigp[:, 4, kc, :],
                lambda m, ps: nc.vector.tensor_add(
                    out=w1_sb[:, m, :], in0=ps, in1=bigp[:, 3, m, :]),
            )
            # t0 = A0 @ w0 ;  out = init + t0
            spmm_pass(
                ats[0],
                lambda kc: w0_sb[:, kc, :],
                lambda m, ps: nc.vector.tensor_add(
                    out=out_sb[:, m, :], in0=ps, in1=bigp[:, 2, m, :]),
            )
            # t1 = A1 @ w1 ;  out += t1 ; stream result out per chunk
            outd_v = outd.rearrange("p (m f) -> p m f", f=FREE)

            def _t1_post(m, ps):
                nc.vector.tensor_add(
                    out=out_sb[:, m, :], in0=ps, in1=out_sb[:, m, :])
                nc.sync.dma_start(out=outd_v[:, m, :], in_=out_sb[:, m, :])

            # panels on the ACT ring, result stores on the SP ring: no
            # store-behind-panel FIFO stalls in the final pass
            spmm_pass(ats[1], lambda kc: w1_sb[:, kc, :], _t1_post,
                      panel_ring=nc.scalar)
            pp_cm.__exit__(None, None, None)
            sp_cm.__exit__(None, None, None)

    nc.compile()
    return nc


def _get_compiled():
    global _COMPILED
    if _COMPILED is None:
        _import_concourse()
        _COMPILED = _build_module()
    return _COMPILED


def _densify_panels(rows, cols, vals):
    """COO -> dense fp16 in matmul panel layout at[m, p, kc, j] = A[m*128+j, kc*128+p]."""
    A = np.zeros((N, N), np.float32)
    np.add.at(A, (np.asarray(rows), np.asarray(cols)), np.asarray(vals))
    at = A.reshape(NM, P, NK, P).transpose(0, 3, 2, 1)
    return np.ascontiguousarray(at, dtype=np.float16)


def kernel(inputs, state, rows0, cols0, vals0, rows1, cols1, vals1,
           weight, biases, output_size):
    global LAST_RESULTS
    _import_concourse()
    from concourse.bass_utils import run_bass_kernel_spmd

    inputs = np.asarray(inputs, dtype=np.float32)
    state = np.asarray(state, dtype=np.float32)
    weight = np.asarray(weight, dtype=np.float32)
    biases = np.asarray(biases, dtype=np.float32)
    B = inputs.shape[0]
    assert B == NCORES * BC

    # ---- host prep: static graph/weight preprocessing + layout ----
    at0 = _densify_panels(rows0, cols0, vals0)
    at1 = _densify_panels(rows1, cols1, vals1)

    W = weight.reshape(P, 5, 64)  # [feat, matrix, out]
    W0, W10, W20, W11, W21 = (W[:, m, :] for m in range(5))
    wcat = np.concatenate(
        [2.0 * W20, W10, W0 - W20 - W21, W11, 2.0 * W21], axis=1
    ).astype(np.float16)
    wcat = np.ascontiguousarray(wcat)

    # feat-major x0 per batch: x0t[b, f, n]
    xin = inputs.reshape(B, N, 64)
    xst = state.reshape(B, N, 64)
    x0t = np.empty((B, P, N), np.float16)
    x0t[:, :64, :] = xin.transpose(0, 2, 1)
    x0t[:, 64:, :] = xst.transpose(0, 2, 1)

    nc = _get_compiled()
    in_maps = [
        {
            "at0": at0,
            "at1": at1,
            "wcat": wcat,
            "x0t": np.ascontiguousarray(x0t[c * BC:(c + 1) * BC]),
        }
        for c in range(NCORES)
    ]
    # The axon terminal occasionally reports NRT_EXEC_UNIT_UNRECOVERABLE on
    # the first execution of a freshly compiled NEFF; a reload retry succeeds.
    last_exc = None
    for _attempt in range(3):
        try:
            res = run_bass_kernel_spmd(nc, in_maps, core_ids=list(range(NCORES)))
            break
        except Exception as e:  # noqa: BLE001
            last_exc = e
            import time
            time.sleep(5.0)
    else:
        raise last_exc
    LAST_RESULTS = res

    out = np.empty((B, N * 64), np.float32)
    for c in range(NCORES):
        r = np.asarray(res.results[c]["out"])  # [P, NM*FREE]
        # r[p, m*256 + bi*64 + f] = out[bi, m*128+p, f]
        out[c * BC:(c + 1) * BC] = (
            r.reshape(P, NM, BC, 64).transpose(2, 1, 0, 3).reshape(BC, N * 64)
        )
    # biases are all zeros in this problem spec, but honor them anyway
    if np.any(biases):
        out += np.tile(biases, N)[None, :]
    return out



# revision 2
# speedup vs baseline: 1.7066x; 1.7066x over previous
"""Trainium2 Bass kernel for nn_DiffusionGraphConv_78374563217429.

Math reformulation (exact algebra):
  reference out = concat_m(x_m) @ W  with  xs = [x0, A0 x0, 2 A0^2 x0 - x0,
                                                 A1 x0, 2 A1^2 x0 - x0]
  Since everything is linear, push W through the recurrence:
      out = x0 @ Wd + sum_s A_s @ (x0 @ W1s + A_s @ (x0 @ 2 W2s))
  with Wd = W0 - W20 - W21.  This shrinks each SpMM application from 128
  features to 64 and removes the big final [B*N,640]@[640,64] matmul.

Implementation: the COO supports are densified host-side (static graph
preprocessing) into fp8-e4m3 [4096,4096] matrices (scaled x16 into the
healthy e4m3 range) laid out in matmul-ready panels; on each core the
recurrence runs as dense TensorE matmuls in DoubleRow fp8 perf mode
(256-deep contraction per instruction) with fp32 PSUM accumulation.
Power-of-2 scales keep every fp8 operand near unit RMS:
  u_s  = x0 @ (2 W2s)          (fp8)
  wt_s = x0 @ (16 W1s)         (fp16)
  w_s  = A8_s @ u_s + wt_s     (fp8;  = 16 w_s_true)
  out  = (A8_s @ w_s) * 2^-8 + init   (init = x0 @ Wd, fp16)
Host emulation of this exact pipeline gives rel err 4.5e-3 (gate 2e-2).

Sharding: data-parallel over batch, 4 batch items per core x 8 cores;
supports/weights replicated.
"""

import os
import sys

import numpy as np

# ---------------------------------------------------------------- constants
P = 128          # partitions
N = 4096         # nodes
NM = 32          # output-node chunks (N / P)
NK = 32          # contraction-node chunks (N / P)
BC = 4           # batch items per core
FREE = BC * 64   # matmul moving free dim for SpMM passes (4 batches x 64 feat)
PW = 320         # P-phase Wcat columns: [u0 | u1 | wt0 | wt1 | init] x 64
NCORES = 8

_COMPILED = None     # cached (nc, ) across kernel() calls
LAST_RESULTS = None  # BassKernelResults of the most recent run (for test.py)


def _import_concourse():
    try:
        import concourse.bass  # noqa: F401
    except ImportError:
        for p in ("/opt/trn_rl_repo", "/root/.axon_site/_ro/trn_rl_repo"):
            if os.path.isdir(p) and p not in sys.path:
                sys.path.insert(0, p)
        import concourse.bass  # noqa: F401
    # bass_utils imports antenv.axon_hooks when tracing is requested; some
    # images lack that module — stub it so BASS_TRACE never crashes the run.
    try:
        import antenv.axon_hooks  # noqa: F401
    except ImportError:
        import types
        mod = types.ModuleType("antenv.axon_hooks")
        mod.get_axon_ntff_profile_hook = lambda: None
        mod.set_axon_ntff_profile_hook = lambda h: None
        sys.modules["antenv.axon_hooks"] = mod


def _build_module():
    """Trace the Bass/Tile module (identical SPMD program for all 8 cores)."""
    import concourse.mybir as mybir
    from concourse import bacc
    from concourse.tile import TileContext

    f8 = mybir.dt.float8e4
    f16 = mybir.dt.float16
    f32 = mybir.dt.float32
    DR = mybir.MatmulPerfMode.DoubleRow
    MULT = mybir.AluOpType.mult
    ADD = mybir.AluOpType.add

    nc = bacc.Bacc("TRN2", target_bir_lowering=False, debug=False,
                   num_devices=NCORES)

    at0 = nc.dram_tensor("at0", [NM, P, NK, P], f8, kind="ExternalInput").ap()
    at1 = nc.dram_tensor("at1", [NM, P, NK, P], f8, kind="ExternalInput").ap()
    x0t = nc.dram_tensor("x0t", [BC, P, N], f16, kind="ExternalInput").ap()
    wcat = nc.dram_tensor("wcat", [P, PW], f16, kind="ExternalInput").ap()
    outd = nc.dram_tensor("out", [P, NM * FREE], f32, kind="ExternalOutput").ap()

    ats = (at0, at1)

    with TileContext(nc) as tc:
        with (
            tc.tile_pool(name="singles", bufs=1) as singles,
            tc.tile_pool(name="trans", bufs=5) as trans,
            tc.tile_pool(name="xp", bufs=2) as xp,
        ):
            wcat_sb = singles.tile([P, PW], f16, name="wcat_sb")
            nc.sync.dma_start(out=wcat_sb, in_=wcat)

            # ---- PE warmup: HAM clock-gate starts at 1.2 GHz and only
            # releases after ~3.4us of sustained PE activity.  Fill the
            # DMA-load window with dummy matmuls so the real work runs
            # at 2.4 GHz from the first instruction.
            wlhs = singles.tile([P, P], f16, name="wlhs")
            wrhs = singles.tile([P, 512], f16, name="wrhs")
            nc.vector.memset(wlhs, 0.0)
            nc.vector.memset(wrhs, 0.0)

            # persistent SBUF buffers
            # bigp8 sections: 0=u0, 1=u1 (fp8, SpMM rhs; layout [p,s,m,b*64+f])
            # bigp16 sections: 0=wt0, 1=wt1, 2=init (fp16)
            bigp8 = singles.tile([P, 2, NM, FREE], f8, name="bigp8")
            bigp16 = singles.tile([P, 3, NM, FREE], f16, name="bigp16")
            w0_sb = singles.tile([P, NK, FREE], f8, name="w0_sb")
            w1_sb = singles.tile([P, NK, FREE], f8, name="w1_sb")
            out_sb = singles.tile([P, NM, FREE], f32, name="out_sb")

            # ---------------- P phase: P_b = x0_b @ Wcat ----------------
            # Wcat col sections: [0:64]=2*W20 (u0), [64:128]=2*W21 (u1),
            # [128:192]=16*W10 (wt0), [192:256]=16*W11 (wt1),
            # [256:320]=Wd (init)
            sp_cm = tc.tile_pool(name="sp", bufs=2, space="PSUM")
            sp = sp_cm.__enter__()
            pp_cm = tc.tile_pool(name="pp", bufs=3, space="PSUM")
            pp = pp_cm.__enter__()
            wps = sp.tile([P, FREE], f32, tag="sp_ps", name="warm_ps")
            for _ in range(30):
                nc.tensor.matmul(wps, wlhs, wrhs[:, :FREE], start=True, stop=True)
            for b in range(BC):
                xt = xp.tile([P, N], f16, tag="xt", name="xt")
                # two half-tile DMAs: the first m-chunks' matmuls only
                # depend on the first half, starting the P phase earlier
                nc.sync.dma_start(out=xt[:, :N // 2], in_=x0t[b, :, :N // 2])
                nc.sync.dma_start(out=xt[:, N // 2:], in_=x0t[b, :, N // 2:])
                for mg in range(NM // 2):
                    ps = pp.tile([P, 2, 512], f32, tag="pp_ps", name="pp_ps")
                    for mi in range(2):
                        m = mg * 2 + mi
                        nc.tensor.matmul(
                            ps[:, mi, :PW],
                            xt[:, m * P:(m + 1) * P],
                            wcat_sb,
                            start=True, stop=True,
                        )
                    # strided evacuation, u-sections to fp8 and wt/init to
                    # fp16; groups alternate DVE/ACT to halve fixed costs
                    if mg % 2 == 0:
                        e8, e16 = nc.vector.tensor_copy, (
                            lambda out, in_: nc.scalar.copy(out=out, in_=in_))
                    else:
                        e16, e8 = nc.vector.tensor_copy, (
                            lambda out, in_: nc.scalar.copy(out=out, in_=in_))
                    e8(
                        out=bigp8[:, :, mg * 2:(mg + 1) * 2, b * 64:(b + 1) * 64],
                        in_=ps[:, :, 0:128].rearrange(
                            "p m (s f) -> p s m f", f=64),
                    )
                    e16(
                        out=bigp16[:, :, mg * 2:(mg + 1) * 2, b * 64:(b + 1) * 64],
                        in_=ps[:, :, 128:PW].rearrange(
                            "p m (s f) -> p s m f", f=64),
                    )

            # ---------------- SpMM passes (fp8 DoubleRow) ----------------
            def spmm_pass(at_ap, rhs_fn, post_fn, panel_ring=None):
                for m in range(NM):
                    panel = trans.tile([P, NK, P], f8, tag="big8k", name="panel")
                    # alternate HWDGE rings (SP / ACT) so panel loads stream
                    # on both queues instead of one FIFO
                    if panel_ring is None:
                        dma_eng = nc.sync if m % 2 == 0 else nc.scalar
                    else:
                        dma_eng = panel_ring
                    dma_eng.dma_start(out=panel, in_=at_ap[m])
                    ps = sp.tile([P, FREE], f32, name="sp_ps")
                    for kp in range(NK // 2):
                        nc.tensor.matmul(
                            ps,
                            panel[:, 2 * kp:2 * kp + 2, :],
                            rhs_fn(kp),
                            start=(kp == 0), stop=(kp == NK // 2 - 1),
                            perf_mode=DR,
                        )
                    post_fn(m, ps)

            # v0 = A8_0 @ u0 ;  w0 = v0 + wt0
            spmm_pass(
                ats[0],
                lambda kp: bigp8[:, 0, 2 * kp:2 * kp + 2, :],
                lambda m, ps: nc.vector.tensor_add(
                    out=w0_sb[:, m, :], in0=ps, in1=bigp16[:, 0, m, :]),
            )
            # v1 = A8_1 @ u1 ;  w1 = v1 + wt1
            spmm_pass(
                ats[1],
                lambda kp: bigp8[:, 1, 2 * kp:2 * kp + 2, :],
                lambda m, ps: nc.vector.tensor_add(
                    out=w1_sb[:, m, :], in0=ps, in1=bigp16[:, 1, m, :]),
            )
            # t0 = A8_0 @ w0 ;  out = t0 * 2^-8 + init
            spmm_pass(
                ats[0],
                lambda kp: w0_sb[:, 2 * kp:2 * kp + 2, :],
                lambda m, ps: nc.vector.scalar_tensor_tensor(
                    out=out_sb[:, m, :], in0=ps, scalar=2.0 ** -8,
                    in1=bigp16[:, 2, m, :], op0=MULT, op1=ADD),
            )
            # t1 = A8_1 @ w1 ;  out += t1 * 2^-8 ; stream result out per chunk
            outd_v = outd.rearrange("p (m f) -> p m f", f=FREE)

            def _t1_post(m, ps):
                nc.vector.scalar_tensor_tensor(
                    out=out_sb[:, m, :], in0=ps, scalar=2.0 ** -8,
                    in1=out_sb[:, m, :], op0=MULT, op1=ADD)
                nc.sync.dma_start(out=outd_v[:, m, :], in_=out_sb[:, m, :])

            # panels on the ACT ring, result stores on the SP ring: no
            # store-behind-panel FIFO stalls in the final pass
            spmm_pass(ats[1], lambda kp: w1_sb[:, 2 * kp:2 * kp + 2, :],
                      _t1_post, panel_ring=nc.scalar)
            pp_cm.__exit__(None, None, None)
            sp_cm.__exit__(None, None, None)

    nc.compile()
    return nc


def _get_compiled():
    global _COMPILED
    if _COMPILED is None:
        _import_concourse()
        _COMPILED = _build_module()
    return _COMPILED


def _f8_dtype():
    import ml_dtypes
    if hasattr(ml_dtypes, "float8_e4m3"):
        return ml_dtypes.float8_e4m3
    return ml_dtypes.float8_e4m3fn


def _densify_panels(rows, cols, vals):
    """COO -> dense fp8 (x16 scaled) panels at[m, p, kc, j] = 16*A[m*128+j, kc*128+p]."""
    A = np.zeros((N, N), np.float32)
    np.add.at(A, (np.asarray(rows), np.asarray(cols)), np.asarray(vals))
    at = (16.0 * A).reshape(NM, P, NK, P).transpose(0, 3, 2, 1)
    return np.ascontiguousarray(at).astype(_f8_dtype())


def kernel(inputs, state, rows0, cols0, vals0, rows1, cols1, vals1,
           weight, biases, output_size):
    global LAST_RESULTS
    _import_concourse()
    from concourse.bass_utils import run_bass_kernel_spmd

    inputs = np.asarray(inputs, dtype=np.float32)
    state = np.asarray(state, dtype=np.float32)
    weight = np.asarray(weight, dtype=np.float32)
    biases = np.asarray(biases, dtype=np.float32)
    B = inputs.shape[0]
    assert B == NCORES * BC

    # ---- host prep: static graph/weight preprocessing + layout ----
    at0 = _densify_panels(rows0, cols0, vals0)
    at1 = _densify_panels(rows1, cols1, vals1)

    W = weight.reshape(P, 5, 64)  # [feat, matrix, out]
    W0, W10, W20, W11, W21 = (W[:, m, :] for m in range(5))
    wcat = np.concatenate(
        [2.0 * W20, 2.0 * W21, 16.0 * W10, 16.0 * W11, W0 - W20 - W21], axis=1
    ).astype(np.float16)
    wcat = np.ascontiguousarray(wcat)

    # feat-major x0 per batch: x0t[b, f, n]
    xin = inputs.reshape(B, N, 64)
    xst = state.reshape(B, N, 64)
    x0t = np.empty((B, P, N), np.float16)
    x0t[:, :64, :] = xin.transpose(0, 2, 1)
    x0t[:, 64:, :] = xst.transpose(0, 2, 1)

    nc = _get_compiled()
    in_maps = [
        {
            "at0": at0,
            "at1": at1,
            "wcat": wcat,
            "x0t": np.ascontiguousarray(x0t[c * BC:(c + 1) * BC]),
        }
        for c in range(NCORES)
    ]
    # The axon terminal occasionally reports NRT_EXEC_UNIT_UNRECOVERABLE on
    # the first execution of a freshly compiled NEFF; a reload retry succeeds.
    last_exc = None
    for _attempt in range(3):
        try:
            res = run_bass_kernel_spmd(nc, in_maps, core_ids=list(range(NCORES)))
            break
        except Exception as e:  # noqa: BLE001
            last_exc = e
            import time
            time.sleep(5.0)
    else:
        raise last_exc
    LAST_RESULTS = res

    out = np.empty((B, N * 64), np.float32)
    for c in range(NCORES):
        r = np.asarray(res.results[c]["out"])  # [P, NM*FREE]
        # r[p, m*256 + bi*64 + f] = out[bi, m*128+p, f]
        out[c * BC:(c + 1) * BC] = (
            r.reshape(P, NM, BC, 64).transpose(2, 1, 0, 3).reshape(BC, N * 64)
        )
    # biases are all zeros in this problem spec, but honor them anyway
    if np.any(biases):
        out += np.tile(biases, N)[None, :]
    return out
